# revision 22
# baseline (speedup 1.0000x reference)
"""Per-Region Normalization (SPADE-like) Trainium2 kernel.

Math (reference):
  nf        = batchnorm(fp) * bn_gamma + bn_beta          # stats over (B,H,W)
  codes     = select(style_codes, mask_codes)             # [B,J,S]
  mu        = relu(codes @ W_mu^T + b_mu)                 # [B,J,S]
  middle    = scatter mu by one-hot masks sg              # [B,S,H,W]
  gamma_avg = conv3x3(middle, W_gamma) + b_gamma
  beta_avg  = conv3x3(middle, W_beta)  + b_beta
  out       = nf * (1 + gamma_avg) + beta_avg

Key algebraic collapse: middle has rank J per image (8 style vectors
scattered by disjoint one-hot masks), so
  conv(middle)[c, p] = sum_{tap, j} G[(tap, j), c] * OH[(tap, j), p]
where G[(tap,j), c] = sum_s W_conv[c, s, tap] * mu[j, s]  (tiny matmuls)
and   OH[(tap,j), p] = sg_padded[j, p + shift(tap)]       (shifted one-hot).
Adding a constant row OH[72, :] = 1 with G[72, c] = bias folds the conv
bias (and the "+1" for gamma) into the same matmul. This turns ~154
GFLOP of conv into one [73,128]^T x [73,512] matmul per output tile.

Sharding over 8 cores: core = b*2 + ch  (b in 0..3, ch = channel half).
Each core handles fp[b, ch*128:(ch+1)*128, :, :] (all pixels).
BatchNorm statistics need a reduction over the 4 cores that share a
channel half -> one 1KB AllReduce over replica groups {0,2,4,6},{1,3,5,7}.
"""

import ml_dtypes
import numpy as np

import concourse.bass as bass  # noqa: F401
import concourse.tile as tile
from concourse import bacc, mybir
from concourse.bass_utils import run_bass_kernel_spmd

B, C, S, H, W, J = 4, 256, 256, 128, 128, 8
BN_EPS = 1e-5
PIX = H * W            # 16384
NSPLIT = 512           # free-dim chunk (one PSUM bank, fp32r full rate)
NCHUNK = PIX // NSPLIT  # 32
F32 = mybir.dt.float32
BF16 = mybir.dt.bfloat16
I32 = mybir.dt.int32

_NC = None


def _build_nc():
    nc = bacc.Bacc(trn_type="TRN2", num_devices=8)

    fp_d = nc.dram_tensor("fp0", [128, PIX], F32, kind="ExternalInput")
    sgp_d = nc.dram_tensor("sgp", [J, H + 2, W + 2], BF16, kind="ExternalInput")
    styT_d = nc.dram_tensor("styT", [2, 128, J + 1], F32, kind="ExternalInput")
    mask_d = nc.dram_tensor("mask", [1, J], I32, kind="ExternalInput")
    wmuT_d = nc.dram_tensor("wmuT", [J * 2, 128, S], F32, kind="ExternalInput")
    bmuT_d = nc.dram_tensor("bmuT", [2, 128, J], F32, kind="ExternalInput")
    wgb_d = nc.dram_tensor("wgb", [2, 128, 9 * 2 * 128], F32, kind="ExternalInput")
    bgb_d = nc.dram_tensor("bgb", [2, 128], F32, kind="ExternalInput")
    bnaff_d = nc.dram_tensor("bnaff", [128, 2], F32, kind="ExternalInput")
    out_d = nc.dram_tensor("out0", [128, PIX], F32, kind="ExternalOutput")

    mult = mybir.AluOpType.mult
    add = mybir.AluOpType.add
    sub = mybir.AluOpType.subtract
    AF = mybir.ActivationFunctionType

    with tile.TileContext(nc) as tc:
        with (
            tc.tile_pool(name="big", bufs=1) as big,
            tc.tile_pool(name="wt", bufs=1) as wt,
            tc.tile_pool(name="sm", bufs=1) as sm,
            tc.tile_pool(name="tails", bufs=3) as tails,
            tc.tile_pool(name="outs", bufs=2) as outs,
            tc.tile_pool(name="ps_set", bufs=1, space="PSUM") as ps_set,
            tc.tile_pool(name="ps_mm", bufs=2, space="PSUM") as ps_mm,
            tc.tile_pool(name="dram", bufs=1, space="DRAM") as dpool,
        ):
            # ---------------- fp load + local BN stats ----------------
            fp_t = big.tile([128, PIX], F32)
            for i in range(4):
                sl = slice(i * 4096, (i + 1) * 4096)
                nc.sync.dma_start(out=fp_t[:, sl], in_=fp_d[:, sl])
            stats = sm.tile([128, NCHUNK, 6], F32)
            for i in range(NCHUNK):
                nc.vector.bn_stats(
                    out=stats[:, i, :], in_=fp_t[:, i * NSPLIT:(i + 1) * NSPLIT]
                )
            mv = sm.tile([128, 2], F32)
            nc.vector.bn_aggr(out=mv, in_=stats)
            # payload = (mean, E[x^2]) so sums are linear across cores
            pay = sm.tile([128, 2], F32)
            nc.vector.tensor_copy(out=pay[:, 0:1], in_=mv[:, 0:1])
            nc.vector.scalar_tensor_tensor(
                out=pay[:, 1:2], in0=mv[:, 0:1], scalar=mv[:, 0:1],
                in1=mv[:, 1:2], op0=mult, op1=add,
            )
            cc_in = dpool.tile([128, 2], F32)
            cc_out = dpool.tile([128, 2], F32)
            nc.gpsimd.dma_start(out=cc_in, in_=pay)
            nc.gpsimd.collective_compute(
                "AllReduce",
                mybir.AluOpType.add,
                replica_groups=[[0, 2, 4, 6], [1, 3, 5, 7]],
                ins=[cc_in.opt()],
                outs=[cc_out.opt()],
            )
            gmv = sm.tile([128, 2], F32)
            nc.gpsimd.dma_start(out=gmv, in_=cc_out)

            bnaff = sm.tile([128, 2], F32)
            nc.sync.dma_start(out=bnaff, in_=bnaff_d[:])
            mean_g = sm.tile([128, 1], F32)
            nc.scalar.mul(out=mean_g, in_=gmv[:, 0:1], mul=0.25)
            msq = sm.tile([128, 1], F32)
            nc.vector.tensor_scalar(
                out=msq, in0=mean_g, scalar1=mean_g[:, 0:1], scalar2=None, op0=mult
            )
            var_g = sm.tile([128, 1], F32)
            nc.vector.scalar_tensor_tensor(
                out=var_g, in0=gmv[:, 1:2], scalar=0.25, in1=msq, op0=mult, op1=sub
            )
            eps_t = sm.tile([128, 1], F32)
            nc.vector.memset(eps_t, BN_EPS)
            std = sm.tile([128, 1], F32)
            nc.scalar.activation(
                out=std, in_=var_g, func=AF.Sqrt, bias=eps_t[:, 0:1]
            )
            inv = sm.tile([128, 1], F32)
            nc.vector.reciprocal(out=inv, in_=std)
            scale_c = sm.tile([128, 1], F32)
            nc.vector.tensor_scalar(
                out=scale_c, in0=inv, scalar1=bnaff[:, 0:1], scalar2=None, op0=mult
            )
            bias_c = sm.tile([128, 1], F32)
            nc.vector.tensor_scalar(
                out=bias_c, in0=mean_g, scalar1=scale_c[:, 0:1], scalar2=-1.0,
                op0=mult, op1=mult,
            )
            nc.vector.tensor_add(out=bias_c, in0=bias_c, in1=bnaff[:, 1:2])

            # ---------------- style code selection (codesT [s, j]) ------
            mi = sm.tile([1, J], I32)
            nc.sync.dma_start(out=mi, in_=mask_d[:])
            mf = sm.tile([1, J], F32)
            nc.vector.tensor_copy(out=mf, in_=mi)
            # broadcast mf across partitions via a rank-1 matmul
            ones_r = sm.tile([1, 128], F32)
            nc.vector.memset(ones_r, 1.0)
            mps = ps_set.tile([128, J], F32, tag="bc")
            nc.tensor.matmul(out=mps, lhsT=ones_r, rhs=mf, start=True, stop=True)
            m_b = sm.tile([128, J], F32)
            nc.scalar.copy(out=m_b, in_=mps)
            om_b = sm.tile([128, J], F32)
            nc.vector.tensor_scalar(
                out=om_b, in0=m_b, scalar1=-1.0, scalar2=1.0, op0=mult, op1=add
            )
            styT = sm.tile([128, 2, J + 1], F32)
            nc.sync.dma_start(out=styT, in_=styT_d[:].rearrange("k p f -> p k f"))
            codesT = sm.tile([128, 2, J], F32)
            ctmp = sm.tile([128, J], F32)
            for k in range(2):
                nc.vector.tensor_tensor(
                    out=ctmp, in0=styT[:, k, 0:J], in1=m_b, op=mult
                )
                nc.vector.tensor_scalar(
                    out=codesT[:, k, :], in0=om_b, scalar1=styT[:, k, J:J + 1],
                    scalar2=None, op0=mult,
                )
                nc.vector.tensor_add(
                    out=codesT[:, k, :], in0=codesT[:, k, :], in1=ctmp
                )

            # ---------------- muT = relu(W_mu @ codes + b_mu) [o, j] ----
            wmuT = wt.tile([128, J * 2, S], F32)
            nc.sync.dma_start(out=wmuT, in_=wmuT_d[:].rearrange("a p f -> p a f"))
            bmuT = sm.tile([128, 2, J], F32)
            nc.sync.dma_start(out=bmuT, in_=bmuT_d[:].rearrange("k p f -> p k f"))
            muT = sm.tile([128, 2, J], F32)
            for oc in range(2):
                mups = ps_set.tile([128, J], F32, tag="mups")
                for j in range(J):
                    for k in range(2):
                        nc.tensor.matmul(
                            out=mups[:, j:j + 1],
                            lhsT=wmuT[:, j * 2 + k, oc * 128:(oc + 1) * 128],
                            rhs=codesT[:, k, j:j + 1],
                            start=(k == 0), stop=(k == 1),
                        )
                nc.vector.tensor_add(
                    out=muT[:, oc, :], in0=mups, in1=bmuT[:, oc, :]
                )
                nc.vector.tensor_scalar_max(
                    out=muT[:, oc, :], in0=muT[:, oc, :], scalar1=0.0
                )

            # ------------- G matrices  gcat [(tap,j)+bias, conv, c] -----
            wgb = wt.tile([128, 2, 9 * 2 * 128], F32)
            nc.sync.dma_start(out=wgb, in_=wgb_d[:].rearrange("k p f -> p k f"))
            gcat = sm.tile([73, 2, 128], F32)
            for tap in range(9):
                fsl = slice(tap * 256, (tap + 1) * 256)
                gt = ps_set.tile([8, 2, 128], F32, tag="gt")
                for k in range(2):
                    nc.tensor.matmul(
                        out=gt, lhsT=muT[:, k, :],
                        rhs=wgb[:, k, fsl],
                        start=(k == 0), stop=(k == 1),
                    )
                gts = sm.tile([8, 2, 128], F32, tag="gts")
                nc.scalar.copy(out=gts, in_=gt)
                nc.sync.dma_start(
                    out=gcat[tap * 8:(tap + 1) * 8, :, :], in_=gts
                )
            bias_t = sm.tile([1, 2, 128], F32)
            nc.sync.dma_start(out=bias_t[:, 0, :], in_=bgb_d[0:1, :])
            nc.sync.dma_start(out=bias_t[:, 1, :], in_=bgb_d[1:2, :])
            nc.vector.tensor_scalar_add(
                out=bias_t[:, 0, :], in0=bias_t[:, 0, :], scalar1=1.0
            )
            nc.sync.dma_start(out=gcat[72:73, :, :], in_=bias_t)
            # hi/lo bf16 split of gcat: ghi + glo == gcat to ~2^-17 rel,
            # so two accumulating bf16 matmuls give ~fp32 accuracy.
            ghi = sm.tile([73, 2, 128], BF16)
            nc.vector.tensor_copy(out=ghi, in_=gcat)
            ghf = sm.tile([73, 2, 128], F32)
            nc.vector.tensor_copy(out=ghf, in_=ghi)
            glf = sm.tile([73, 2, 128], F32)
            nc.vector.tensor_sub(out=glf, in0=gcat, in1=ghf)
            glo = sm.tile([73, 2, 128], BF16)
            nc.vector.tensor_copy(out=glo, in_=glf)

            # ---------------- shifted one-hot matrix OH -----------------
            oh = big.tile([73, PIX], BF16)
            nc.gpsimd.memset(oh[64:73, :], 1.0)
            for ky in range(3):
                for kx in range(3):
                    tap = ky * 3 + kx
                    dest = oh[tap * 8:(tap + 1) * 8, :].rearrange(
                        "p (h w) -> p h w", h=H
                    )
                    nc.sync.dma_start(
                        out=dest, in_=sgp_d[:, ky:ky + H, kx:kx + W]
                    )

            # ---------------- assembly + elementwise tail ---------------
            # output staged in [128, 4096] tiles so stores go out as 2MB DMAs
            OSPAN = 4096
            ostg = None
            for n in range(NCHUNK):
                nsl = slice(n * NSPLIT, (n + 1) * NSPLIT)
                if n % (OSPAN // NSPLIT) == 0:
                    ostg = outs.tile([128, OSPAN], F32)
                osl = slice((n % (OSPAN // NSPLIT)) * NSPLIT,
                            (n % (OSPAN // NSPLIT) + 1) * NSPLIT)
                gps = ps_mm.tile([128, NSPLIT], F32, tag="gps")
                bps = ps_mm.tile([128, NSPLIT], F32, tag="bps")
                nc.tensor.matmul(
                    out=gps, lhsT=ghi[:, 0, :], rhs=oh[:, nsl],
                    start=True, stop=False,
                )
                nc.tensor.matmul(
                    out=gps, lhsT=glo[:, 0, :], rhs=oh[:, nsl],
                    start=False, stop=True,
                )
                nc.tensor.matmul(
                    out=bps, lhsT=ghi[:, 1, :], rhs=oh[:, nsl],
                    start=True, stop=False,
                )
                nc.tensor.matmul(
                    out=bps, lhsT=glo[:, 1, :], rhs=oh[:, nsl],
                    start=False, stop=True,
                )
                nf = tails.tile([128, NSPLIT], F32)
                nc.scalar.activation(
                    out=nf, in_=fp_t[:, nsl], func=AF.Identity,
                    bias=bias_c[:, 0:1], scale=scale_c[:, 0:1],
                )
                nc.vector.tensor_tensor(
                    out=ostg[:, osl], in0=nf, in1=gps, op=mult
                )
                nc.vector.tensor_tensor(
                    out=ostg[:, osl], in0=ostg[:, osl], in1=bps, op=add
                )
                if (n + 1) % (OSPAN // NSPLIT) == 0:
                    dsl = slice((n + 1) * NSPLIT - OSPAN, (n + 1) * NSPLIT)
                    nc.sync.dma_start(out=out_d[:, dsl], in_=ostg)

    nc.compile()
    return nc


def get_nc():
    global _NC
    if _NC is None:
        _NC = _build_nc()
    return _NC


def _shard_inputs(fp, sg, style_codes, mask_codes, bn_gamma, bn_beta,
                  W_mu, b_mu, W_gamma, b_gamma, W_beta, b_beta):
    f4 = np.float32
    in_maps = []
    wmuT = np.ascontiguousarray(
        W_mu.transpose(0, 2, 1).reshape(J * 2, 128, S).astype(f4)
    )
    bmuT = np.ascontiguousarray(b_mu.T.reshape(2, 128, J).astype(f4))
    for core in range(8):
        b, ch = core // 2, core % 2
        csl = slice(ch * 128, (ch + 1) * 128)
        fp_c = np.ascontiguousarray(fp[b, csl].reshape(128, PIX).astype(f4))
        # one-hot 0/1 values are exactly representable in bf16
        sgp = np.zeros((J, H + 2, W + 2), ml_dtypes.bfloat16)
        sgp[:, 1:H + 1, 1:W + 1] = sg[b].astype(ml_dtypes.bfloat16)
        styT = np.ascontiguousarray(
            style_codes[b].T.reshape(2, 128, J + 1).astype(f4)
        )
        mask = np.ascontiguousarray(
            mask_codes[b].reshape(1, J).astype(np.int32)
        )
        wg = W_gamma[csl].transpose(1, 2, 3, 0)   # [s, ky, kx, c]
        wb = W_beta[csl].transpose(1, 2, 3, 0)
        wgb = np.ascontiguousarray(
            np.stack([wg, wb], axis=3).reshape(2, 128, 9 * 2 * 128).astype(f4)
        )
        bgb = np.ascontiguousarray(
            np.stack([b_gamma[csl], b_beta[csl]]).astype(f4)
        )
        bnaff = np.ascontiguousarray(
            np.stack([bn_gamma[csl], bn_beta[csl]], axis=1).astype(f4)
        )
        in_maps.append({
            "fp0": fp_c, "sgp": sgp, "styT": styT, "mask": mask,
            "wmuT": wmuT, "bmuT": bmuT, "wgb": wgb, "bgb": bgb,
            "bnaff": bnaff,
        })
    return in_maps


def kernel(fp, sg, style_codes, mask_codes, bn_gamma, bn_beta,
           W_mu, b_mu, W_gamma, b_gamma, W_beta, b_beta, **run_kwargs):
    args = [np.asarray(a) for a in (
        fp, sg, style_codes, mask_codes, bn_gamma, bn_beta,
        W_mu, b_mu, W_gamma, b_gamma, W_beta, b_beta)]
    in_maps = _shard_inputs(*args)
    nc = get_nc()
    res = run_bass_kernel_spmd(nc, in_maps, list(range(8)), **run_kwargs)
    out = np.empty((B, C, H, W), np.float32)
    for core in range(8):
        b, ch = core // 2, core % 2
        out[b, ch * 128:(ch + 1) * 128] = (
            res.results[core]["out0"].reshape(128, H, W)
        )
    if run_kwargs:
        kernel.last_result = res
    return out
